# revision 10
# baseline (speedup 1.0000x reference)
"""Trainium2 Bass kernel for nn_CSPCompBlock (dense depthwise-conv CSP block).

Strategy (pure data parallelism, one batch image per NeuronCore):
  - Each of the 3 "pseudo" stages = 4 depthwise 3-tap convs + BN + residual ReLU.
  - Depthwise conv as TensorEngine matmuls: lhsT = diagonal weight matrix
    (per-channel tap weight on the diagonal), rhs = shifted image tile
    (shift = free-dim AP offset), 3 taps accumulate in one PSUM group.
    Residual added via an extra identity matmul into the same PSUM group.
    BN folded into conv4's diagonal weights + bias.
  - fp32r matmul dtype: full PE rate with ~2^-11 rounding (measured ~1e-4
    end-to-end absmax error vs fp32 reference). fp32r matmul dst must be a
    contiguous PSUM range: column taps read zero-halo cols (tile width 132)
    instead of clipping, row taps clip at image edges (dst stays contiguous).
  - Stage-to-stage "transposes" ([C,(H,W)] -> [W,(C,H)] -> [H,(C,W)]) run
    on-chip: PE transpose-mode 128x128 tiles into PSUM, copied back to a
    full-image SBUF tile. Output block2 is exactly stage3's input layout,
    so it is DMA'd straight out with 512B-contiguous descriptors.
"""
import sys

for _p in ("/opt/trn_rl_repo", "/opt/pypackages"):
    if _p not in sys.path:
        sys.path.insert(0, _p)

import numpy as np

import concourse.bacc as bacc
import concourse.bass as bass
import concourse.mybir as mybir
import concourse.tile as tile
from concourse.bass_utils import run_bass_kernel_spmd

F32 = mybir.dt.float32
F32R = mybir.dt.float32r
RELU = mybir.ActivationFunctionType.Relu
ADD = mybir.AluOpType.add
MAX = mybir.AluOpType.max

P = 128          # partitions = per-stage channel dim (C, W, H resp.)
S = 128          # spatial extent (all dims are 128)
HALO = 2         # zero cols each side of work tiles
SW = S + 2 * HALO
BAND = 16        # output rows per band
NBAND = S // BAND
GROW = 4         # psum-group rows (4*128 = 512 = one PSUM bank)
N_MAT = 37       # 3 stages * 4 convs * 3 taps + identity
IDENT = 36
EPS = 1e-3


class _Epi:
    """Round-robin ACT/DVE epilogue + copy dispatcher."""

    def __init__(self, nc):
        self.nc = nc
        self.i = 0

    def relu_bias(self, out_ap, psum_ap, bias_ap):
        # out = relu(psum + bias)
        self.i += 1
        if self.i % 2 == 0:
            self.nc.scalar.activation(out_ap, psum_ap, RELU, bias=bias_ap, scale=1.0)
        else:
            self.nc.vector.tensor_scalar(out_ap, psum_ap, bias_ap, 0.0,
                                         op0=ADD, op1=MAX)

    def copy(self, out_ap, psum_ap):
        self.i += 1
        if self.i % 2 == 0:
            self.nc.scalar.copy(out_ap, psum_ap)
        else:
            self.nc.vector.tensor_copy(out_ap, psum_ap)


def _conv_band(nc, epi, psum_pool, diags, biases, mat_base, bias_idx,
               in_tile, in_base, in_lo, in_hi, in_halo,
               out_tile, out_base, out_lo, out_hi, out_halo,
               axis, dil, resid=None):
    """One depthwise conv over a band of rows, via diag matmuls.

    in_tile: [P, rows, SW or S]; tile row i <-> absolute row (in_base+i);
    in_halo: True if tile has HALO zero cols each side (width SW).
    [in_lo, in_hi): valid absolute row range of the input.
    axis 'row': taps shift across tile rows (clipped at [in_lo, in_hi)).
    axis 'col': taps shift within rows (into zero halo; in_halo required).
    resid: (tile, base, halo) whose rows are added via identity matmul.
    PSUM dst of every matmul is a contiguous range (fp32r requirement).
    """
    ioff = HALO if in_halo else 0
    ooff = HALO if out_halo else 0
    taps = ((1, 0), (0, -dil), (2, dil))  # center first: carries start=True
    if axis == 'col':
        assert in_halo
    groups = []
    g0 = out_lo
    while g0 < out_hi:
        g1 = min(g0 + GROW, out_hi)
        groups.append((g0, g1, psum_pool.tile([P, GROW * S], F32, tag="psum",
                                              name="psg")))
        g0 = g1

    emissions = [[] for _ in groups]
    for ti, (t, off) in enumerate(taps):
        lhsT = diags[:, (mat_base + t) * P:(mat_base + t + 1) * P]
        for gi, (g0, g1, ps) in enumerate(groups):
            if axis == 'row':
                s0 = max(g0 + off, in_lo)
                s1 = min(g1 + off, in_hi)
                if s1 <= s0:
                    continue
                rhs = in_tile[:, s0 - in_base:s1 - in_base, ioff:ioff + S]
                out = ps[:, (s0 - off - g0) * S:(s1 - off - g0) * S]
            else:
                rhs = in_tile[:, g0 - in_base:g1 - in_base,
                              ioff + off:ioff + off + S]
                out = ps[:, 0:(g1 - g0) * S]
            emissions[gi].append((ti, lhsT, out, rhs))
    if resid is not None:
        ident = diags[:, IDENT * P:(IDENT + 1) * P]
        r_tile, r_base, r_halo = resid
        roff = HALO if r_halo else 0
        for gi, (g0, g1, ps) in enumerate(groups):
            rhs = r_tile[:, g0 - r_base:g1 - r_base, roff:roff + S]
            emissions[gi].append((3, ident, ps[:, 0:(g1 - g0) * S], rhs))

    # Emit tap-major (weight-load batching); start/stop flags per group.
    flat = []
    for gi, ems in enumerate(emissions):
        assert ems and ems[0][0] == 0, "center tap must exist"
        for j, (ti, lhsT, out, rhs) in enumerate(ems):
            flat.append((ti, gi, lhsT, out, rhs, j == 0, j == len(ems) - 1))
    flat.sort(key=lambda e: (e[0], e[1]))
    for ti, gi, lhsT, out, rhs, is_start, is_stop in flat:
        nc.tensor.matmul(out, lhsT, rhs, start=is_start, stop=is_stop)

    bias_ap = biases[:, bias_idx:bias_idx + 1]
    for gi, (g0, g1, ps) in enumerate(groups):
        epi.relu_bias(
            out_tile[:, g0 - out_base:g1 - out_base, ooff:ooff + S],
            ps[:, 0:(g1 - g0) * S], bias_ap)


def _stage(nc, tc, epi, psum_pool, psum_t, diags, biases, identf, stage,
           pw, in_full, x_in, out3, in2_full, in3_full):
    """Emit one pseudo stage (8 bands). stage: 0, 1, or 2."""
    mb = stage * 12
    bb = stage * 4
    # persistent work tiles (halo cols pre-zeroed)
    c1 = pw.tile([P, BAND + 4, SW], F32R, tag=f"c1_{stage}", name="c1")
    c2 = pw.tile([P, BAND + 4, SW], F32R, tag=f"c2_{stage}", name="c2")
    c3 = pw.tile([P, BAND, SW], F32R, tag=f"c3_{stage}", name="c3")
    oo = pw.tile([P, BAND, S], F32, tag=f"oo_{stage}", name="oo")
    for t in (c1, c2, c3):
        nc.vector.memset(t[:, :, 0:HALO].bitcast(F32), 0.0)
        nc.vector.memset(t[:, :, HALO + S:SW].bitcast(F32), 0.0)
    if stage == 0:
        xts = [pw.tile([P, BAND + 6, SW], F32R, tag=f"xt{i}", name="xt")
               for i in range(2)]
        for t in xts:
            nc.vector.memset(t[:].bitcast(F32), 0.0)

    for bi in range(NBAND):
        r0 = bi * BAND
        if stage == 0:
            in_lo, in_hi = max(0, r0 - 3), min(S, r0 + BAND + 3)
            in_base = r0 - 3
            xt = xts[bi % 2]
            nc.sync.dma_start(
                xt[:, in_lo - in_base:in_hi - in_base, HALO:HALO + S],
                x_in[:, in_lo:in_hi, :].bitcast(F32R))
            src, s_base, s_lo, s_hi, s_halo = xt, in_base, in_lo, in_hi, True
        else:
            src, s_base, s_lo, s_hi, s_halo = in_full, 0, 0, S, False

        c_lo, c_hi = max(0, r0 - 2), min(S, r0 + BAND + 2)
        c_base = r0 - 2
        _conv_band(nc, epi, psum_pool, diags, biases, mb + 0, bb + 0,
                   src, s_base, s_lo, s_hi, s_halo,
                   c1, c_base, c_lo, c_hi, True, 'row', 1)
        _conv_band(nc, epi, psum_pool, diags, biases, mb + 3, bb + 1,
                   c1, c_base, c_lo, c_hi, True,
                   c2, c_base, c_lo, c_hi, True, 'col', 1)
        _conv_band(nc, epi, psum_pool, diags, biases, mb + 6, bb + 2,
                   c2, c_base, c_lo, c_hi, True,
                   c3, r0, r0, r0 + BAND, True, 'row', 2)
        _conv_band(nc, epi, psum_pool, diags, biases, mb + 9, bb + 3,
                   c3, r0, r0, r0 + BAND, True,
                   oo, r0, r0, r0 + BAND, False, 'col', 2,
                   resid=(src, s_base, s_halo))

        if stage == 0:
            nc.sync.dma_start(out3[0:P, r0:r0 + BAND, :], oo[:])
            for hi in range(BAND):
                pt = psum_t.tile([P, P], F32, tag="pt", name="pt")
                nc.tensor.transpose(pt[:], oo[:, hi, :], identf[:])
                epi.copy(in2_full[:, :, r0 + hi], pt[:])
        elif stage == 1:
            for ci in range(BAND):
                pt = psum_t.tile([P, P], F32, tag="pt", name="pt")
                nc.tensor.transpose(pt[:], oo[:, ci, :], identf[:])
                epi.copy(in3_full[:, r0 + ci, :], pt[:])
        else:
            nc.sync.dma_start(
                out3[2 * P + r0:2 * P + r0 + BAND, :, :]
                .rearrange("c h w -> h c w"),
                oo[:])


def _build_nc():
    nc = bacc.Bacc(None, target_bir_lowering=False)
    x_in = nc.dram_tensor("x", [P, S, S], F32, kind="ExternalInput")
    diags_in = nc.dram_tensor("diags", [P, N_MAT * P], F32, kind="ExternalInput")
    biases_in = nc.dram_tensor("biases", [P, 12], F32, kind="ExternalInput")
    out3 = nc.dram_tensor("out3", [3 * P, S, S], F32, kind="ExternalOutput")

    with tile.TileContext(nc) as tc:
        with tc.tile_pool(name="consts", bufs=1) as cpool, \
             tc.tile_pool(name="full2", bufs=1) as p_in2, \
             tc.tile_pool(name="full3", bufs=1) as p_in3, \
             tc.tile_pool(name="psum", bufs=4, space="PSUM") as psum_pool, \
             tc.tile_pool(name="psum_t", bufs=3, space="PSUM") as psum_t:

            diags = cpool.tile([P, N_MAT * P], F32R, tag="diags")
            biases = cpool.tile([P, 12], F32, tag="biases")
            identf = cpool.tile([P, P], F32, tag="identf")
            nc.sync.dma_start(diags[:], diags_in[:].bitcast(F32R))
            nc.sync.dma_start(biases[:], biases_in[:])
            nc.sync.dma_start(identf[:], diags_in[:, IDENT * P:(IDENT + 1) * P])

            in2_full = p_in2.tile([P, S, S], F32R, tag="in2")   # [w, c, h]
            epi = _Epi(nc)

            # Phase 1: stage 1 (partitions=C, rows=H, cols=W)
            with tc.tile_pool(name="s1work", bufs=1) as pw:
                _stage(nc, tc, epi, psum_pool, psum_t, diags, biases, identf,
                       0, pw, None, x_in, out3, in2_full, None)

            # Phase 2: stage 2 (partitions=W, rows=C, cols=H)
            in3_full = p_in3.tile([P, S, S], F32R, tag="in3")   # [h, c, w]
            with tc.tile_pool(name="s2work", bufs=1) as pw:
                _stage(nc, tc, epi, psum_pool, psum_t, diags, biases, identf,
                       1, pw, in2_full, None, out3, None, in3_full)

            # block2 = in3_full[h, c, w] -> out3[P + c, h, w]
            nc.sync.dma_start(
                out3[P:2 * P, :, :].rearrange("c h w -> h c w"),
                in3_full[:].bitcast(F32))

            # Phase 3: stage 3 (partitions=H, rows=C, cols=W)
            with tc.tile_pool(name="s3work", bufs=1) as pw:
                _stage(nc, tc, epi, psum_pool, psum_t, diags, biases, identf,
                       2, pw, in3_full, None, out3, None, None)
    nc.finalize()
    return nc


def _prep_weights(inputs):
    """Build diag matrices [P, N_MAT*P] and biases [P, 12] (BN folded)."""
    diags = np.zeros((P, N_MAT * P), np.float32)
    biases = np.zeros((P, 12), np.float32)
    idx = np.arange(P)
    for s, pfx in enumerate(("hw", "ch", "cw")):
        ws = [np.asarray(inputs[f"{pfx}_w{j}"]).reshape(P, 3) for j in (1, 2, 3, 4)]
        bs = [np.asarray(inputs[f"{pfx}_b{j}"]).astype(np.float32) for j in (1, 2, 3, 4)]
        g = np.asarray(inputs[f"{pfx}_g"])
        bt = np.asarray(inputs[f"{pfx}_bt"])
        m = np.asarray(inputs[f"{pfx}_m"])
        v = np.asarray(inputs[f"{pfx}_v"])
        scale = (g / np.sqrt(v + EPS)).astype(np.float32)
        for k in range(4):
            w = ws[k].astype(np.float32)
            if k == 3:
                w = w * scale[:, None]
                biases[:, s * 4 + k] = (bs[3] - m) * scale + bt
            else:
                biases[:, s * 4 + k] = bs[k]
            for t in range(3):
                j = s * 12 + k * 3 + t
                diags[idx, j * P + idx] = w[:, t]
    diags[idx, IDENT * P + idx] = 1.0
    return diags, biases


_NC_CACHE = None
TRACE = False            # set True (e.g. from test.py) to capture an NTFF profile
LAST_RESULT = None       # BassKernelResults of the most recent kernel() call


def _get_nc():
    global _NC_CACHE
    if _NC_CACHE is None:
        _NC_CACHE = _build_nc()
    return _NC_CACHE


def kernel(**inputs):
    global LAST_RESULT
    x = np.asarray(inputs["x"], np.float32)          # [8, 128, 128, 128]
    B = x.shape[0]
    diags, biases = _prep_weights(inputs)
    nc = _get_nc()
    in_maps = [{"x": np.ascontiguousarray(x[b]), "diags": diags,
                "biases": biases} for b in range(B)]
    res = run_bass_kernel_spmd(nc, in_maps, core_ids=list(range(B)),
                               trace=TRACE)
    LAST_RESULT = res
    out = np.empty((B, 4 * P, S, S), np.float32)
    out[:, :P] = x
    for b in range(B):
        out[b, P:] = res.results[b]["out3"]
    return out
